# revision 21
# baseline (speedup 1.0000x reference)
"""Cross-layer transcoder kernel for 8 TRN2 NeuronCores.

Sharding: d_transcoder (F=4096) is split 8 ways (512 features per core).
Each core encodes all tokens against its feature slice, computes partial
cross-layer reconstructions for every target layer, and a chunked
ReduceScatter sums the partials; rank i receives target layer i
([B, D] per core, L == n_cores == 8).  The decoder bias is folded into
the pre-RS partial on the owning core (per-core masked bias tensor), so
the post-RS step is a plain DRAM->DRAM copy into the output.

Compute dtype: bf16 operands with fp32 PSUM accumulation (1 cycle/row on
the PE); partials and the ReduceScatter in bf16.

Perf structure (v7): the kernel is PE-bound at a GPIO-throttled 13/16
clock; the exposed time is startup plus the final ReduceScatter.  The
decode is chunked by OUTPUT COLUMNS, not just tokens: three
[1536-token x 256-col] body chunks, then four [512-token x 192-col]
tail chunks.  Column chunking streams W_dec exactly once (~42 GB/s
body, ~125 GB/s tail) — token-windowing alone must re-stream all of
W_dec per window, and that stream loses HBM arbitration against the
concurrent ReduceScatter and starves the PE (10-30 us stalls per
window in earlier revisions).  Each chunk completes an independently
ReduceScatter-able piece; the final exposed RS is [8,512,192] = 1.6 MB
(~21 us) instead of 6.3 MB (~100 us).  W_dec is host-packed
chunk-major so every tile load is one fully-contiguous DMA per
partition.  Queue routing: the Sync HWDGE ring carries ONLY weight
loads (a store waiting on its data-ready semaphore blocks the whole
ring behind it); x loads and rs_in stores ride the Act (scalar) ring;
collectives and post-RS copies ride the GpSimd (SWDGE) queue.  PSUM
runs a single [128,256] tag with 16 rotating buffers (12 live token
subtiles per target layer + 4 slack so consecutive layers pipeline).
"""

import numpy as np
import ml_dtypes

L, B, D, F = 8, 2048, 768, 4096
NCORES = 8
FL = F // NCORES          # 512 features per core
AF = FL // 128            # 4 f-tiles per core
DT = D // 128             # 6 d-tiles
EH = 1024                 # encode token chunk per x DMA
# decode chunking (see module docstring)
BODY_TOK = 1536           # body token range [0, 1536)
BODY_NS = BODY_TOK // 128  # 12 token subtiles
BDC = 256                 # body D-chunk width
NBDC = D // BDC           # 3 body chunks
TAIL_B0 = BODY_TOK        # tail token range start
TAIL_TOK = B - TAIL_B0    # 512
TAIL_NS = TAIL_TOK // 128  # 4 token subtiles
TDC = 192                 # tail D-chunk width
NTDC = D // TDC           # 4 tail chunks
W_SCALE = 64.0            # host multiplies W_dec by this before e3m4
F_SCALE = 2.0             # encode ReLU folds this into the fp8 feats
DESCALE = 1.0 / (W_SCALE * F_SCALE)

_COMPILED_NC = None


def _build_nc():
    import concourse.mybir as mybir
    import concourse.tile as tile
    from concourse import bacc

    dt = mybir.dt
    nc = bacc.Bacc("TRN2", target_bir_lowering=False, debug=False,
                   num_devices=NCORES)

    # all weight tensors host-packed for fully-contiguous tile loads
    xtp = nc.dram_tensor("xtp", [L, B // EH, 2, 128, DT, EH // 2],
                         dt.bfloat16, kind="ExternalInput").ap()
    wencp = nc.dram_tensor("wencp", [L, 2, 128, DT // 2, FL],
                           dt.bfloat16, kind="ExternalInput").ap()
    # benc[:, 0:L*AF] plain, benc[:, L*AF:] pre-scaled by F_SCALE
    benc = nc.dram_tensor("benc", [128, 2 * L * AF], dt.float32,
                          kind="ExternalInput").ap()
    wdecb = nc.dram_tensor("wdecb", [NBDC, L, L, 128, AF, BDC],
                           dt.bfloat16, kind="ExternalInput").ap()
    wdect = nc.dram_tensor("wdect", [NTDC, L, L, 128, AF, TDC],
                           dt.float8e3, kind="ExternalInput").ap()
    bdec = nc.dram_tensor("bdec", [L, 128, D], dt.bfloat16, kind="ExternalInput").ap()
    out = nc.dram_tensor("out", [B, D], dt.bfloat16, kind="ExternalOutput").ap()

    RELU = mybir.ActivationFunctionType.Relu
    MULT = mybir.AluOpType.mult
    ADD = mybir.AluOpType.add

    with tile.TileContext(nc) as tc:
        with (
            tc.tile_pool(name="consts", bufs=1) as consts,
            tc.tile_pool(name="featp", bufs=L * AF) as featp,
            tc.tile_pool(name="feat8p", bufs=L * AF) as feat8p,
            tc.tile_pool(name="dram", bufs=1, space="DRAM") as dram,
        ):
            benc_t = consts.tile([128, 2 * L * AF], dt.float32, tag="benc_t")
            nc.sync.dma_start(benc_t[:], benc)
            bdec_t = consts.tile([128, L, D], dt.bfloat16, tag="bdec_t")
            nc.gpsimd.dma_start(bdec_t[:], bdec.rearrange("l p d -> p l d"))

            # bf16 feats cover tokens [0, TAIL_B0); tail tokens live
            # only in the fp8 copies (scaled by F_SCALE at encode time)
            feats = [
                [featp.tile([128, TAIL_B0], dt.bfloat16, name=f"feat_{l}_{a}",
                            tag="feat", bufs=L * AF) for a in range(AF)]
                for l in range(L)
            ]
            feats8 = [
                [feat8p.tile([128, TAIL_TOK], dt.float8e3,
                             name=f"feat8_{l}_{a}", tag="feat8",
                             bufs=L * AF) for a in range(AF)]
                for l in range(L)
            ]

            HB = BODY_TOK // 2  # 768 tokens per body range
            rs_in = (
                [dram.tile([L, HB, BDC], dt.bfloat16,
                           name=f"rs_in_b{w}", tag=f"rsinb{w}")
                 for w in range(2 * NBDC)]
                + [dram.tile([L, TAIL_TOK, TDC], dt.bfloat16,
                             name=f"rs_in_t{t}", tag=f"rsint{t}")
                   for t in range(NTDC)]
            )
            rs_out = (
                [dram.tile([HB, BDC], dt.bfloat16,
                           name=f"rs_out_b{w}", tag=f"rsoutb{w}")
                 for w in range(2 * NBDC)]
                + [dram.tile([TAIL_TOK, TDC], dt.bfloat16,
                             name=f"rs_out_t{t}", tag=f"rsoutt{t}")
                   for t in range(NTDC)]
            )

            # tiny warmup collective: absorbs ncfw/link first-call cost
            # during encode so the first real RS runs at steady-state
            warm_in = dram.tile([L, 128], dt.bfloat16, tag="warmin")
            warm_out = dram.tile([128], dt.bfloat16, tag="warmout")
            nc.gpsimd.collective_compute(
                "ReduceScatter", mybir.AluOpType.add,
                replica_groups=[list(range(NCORES))],
                ins=[warm_in.opt()], outs=[warm_out.opt()])

            def run_rs(w):
                nc.gpsimd.collective_compute(
                    "ReduceScatter", mybir.AluOpType.add,
                    replica_groups=[list(range(NCORES))],
                    ins=[rs_in[w].opt()], outs=[rs_out[w].opt()])
                if w < 2 * NBDC:
                    r, c = w // NBDC, w % NBDC
                    nc.gpsimd.dma_start(
                        out[r * HB:(r + 1) * HB, c * BDC:(c + 1) * BDC],
                        rs_out[w][:])
                else:
                    d0 = (w - 2 * NBDC) * TDC
                    nc.gpsimd.dma_start(
                        out[TAIL_B0:B, d0:d0 + TDC], rs_out[w][:])

            # ---- Phase E: encode all layers/tokens; feats stay in SBUF ----
            with (
                tc.tile_pool(name="encp", bufs=2) as encp,
                tc.tile_pool(name="pep", bufs=4, space="PSUM") as pep,
            ):
                for l in range(L):
                    wenc_t = encp.tile([128, DT, FL], dt.bfloat16,
                                       tag="wenc_t", bufs=2, name=f"wenc_{l}")
                    for q in range(2):
                        nc.sync.dma_start(
                            wenc_t[:, q * (DT // 2):(q + 1) * (DT // 2), :],
                            wencp[l, q])
                    for h in range(B // EH):
                        xt_t = encp.tile([128, DT, EH], dt.bfloat16,
                                         tag="xt_t", bufs=2, name=f"xt_{l}_{h}")
                        for q in range(2):
                            nc.scalar.dma_start(
                                xt_t[:, :, q * (EH // 2):(q + 1) * (EH // 2)],
                                xtp[l, h, q])
                        for a in range(AF):
                            for c in range(EH // 512):
                                ps = pep.tile([128, 512], dt.float32,
                                              tag="pe", bufs=4,
                                              name=f"pe_{l}_{h}_{a}_{c}")
                                for k in range(DT):
                                    nc.tensor.matmul(
                                        ps[:],
                                        wenc_t[:, k, a * 128:(a + 1) * 128],
                                        xt_t[:, k, c * 512:(c + 1) * 512],
                                        start=(k == 0), stop=(k == DT - 1))
                                boff = h * EH + c * 512
                                ca = l * AF + a
                                if boff < TAIL_B0:
                                    nc.scalar.activation(
                                        feats[l][a][:, boff:boff + 512],
                                        ps[:], RELU,
                                        bias=benc_t[:, ca:ca + 1])
                                else:
                                    # tail tokens: fp8 copy only, F_SCALE
                                    # folded in (relu commutes with
                                    # positive scaling)
                                    nc.scalar.activation(
                                        feats8[l][a][:, boff - TAIL_B0:
                                                     boff - TAIL_B0 + 512],
                                        ps[:], RELU,
                                        bias=benc_t[:, L * AF + ca:
                                                    L * AF + ca + 1],
                                        scale=F_SCALE)

            # ---- Phase D: cross-layer decode + chunked ReduceScatter ----
            with (
                tc.tile_pool(name="decp", bufs=16) as decp,
                tc.tile_pool(name="outp", bufs=12) as outp,
                tc.tile_pool(name="pdp", bufs=8, space="PSUM") as pdp,
            ):
                def chunk(w, b0, ns, dc, d0, wsrc, f8=False):
                    """One [ns*128 tokens x dc cols] decode chunk + its RS.

                    ns <= 6: PSUM bank tiles are one-per-subtile (a
                    matmul start zeroes its whole bank, so banks cannot
                    be shared between accumulation chains)."""
                    for j in range(L):
                        pd = [pdp.tile([128, dc], dt.float32, tag="pd",
                                       bufs=8, name=f"pd_{w}_{j}_{s}")
                              for s in range(ns)]
                        for l in range(j + 1):
                            wd = decp.tile([128, AF, dc],
                                           dt.float8e3 if f8 else dt.bfloat16,
                                           tag=f"wd{dc}", bufs=16,
                                           name=f"wd_{w}_{j}_{l}")
                            nc.sync.dma_start(wd[:], wsrc[l, j])
                            st = (l == 0)
                            sp = (l == j)
                            for a in range(AF):
                                for s in range(ns):
                                    if f8:
                                        lhsT = feats8[l][a][
                                            :, s * 128:(s + 1) * 128]
                                    else:
                                        lhsT = feats[l][a][
                                            :, b0 + s * 128:
                                            b0 + (s + 1) * 128]
                                    nc.tensor.matmul(
                                        pd[s][:], lhsT, wd[:, a, :],
                                        start=(st and a == 0),
                                        stop=(sp and a == AF - 1))
                        for s in range(ns):
                            ot = outp.tile([128, dc], dt.bfloat16, tag="ot",
                                           bufs=12, name=f"ot_{w}_{j}_{s}")
                            if f8:
                                nc.vector.scalar_tensor_tensor(
                                    ot[:], pd[s][:], DESCALE,
                                    bdec_t[:, j, d0:d0 + dc], MULT, ADD)
                            else:
                                nc.vector.tensor_add(
                                    ot[:], pd[s][:], bdec_t[:, j, d0:d0 + dc])
                            nc.scalar.dma_start(
                                rs_in[w][j, s * 128:(s + 1) * 128, :], ot[:])
                    run_rs(w)

                # three small tail chunks FIRST (the RS stream starts
                # ~60us earlier, shrinking the backlog on slow-collective
                # runs), then the body (2 token ranges x 3 D-chunks, both
                # ranges reuse the same packed W_dec chunk tensors), and
                # one small tail chunk LAST so the only exposed RS is
                # 1.6 MB.
                for t in range(NTDC - 1):
                    chunk(2 * NBDC + t, TAIL_B0, TAIL_NS, TDC, t * TDC,
                          wdect[t], f8=True)
                for r in range(2):
                    for c in range(NBDC):
                        chunk(r * NBDC + c, r * (BODY_TOK // 2),
                              BODY_NS // 2, BDC, c * BDC, wdecb[c])
                t = NTDC - 1
                chunk(2 * NBDC + t, TAIL_B0, TAIL_NS, TDC, t * TDC,
                      wdect[t], f8=True)

    nc.compile()
    return nc


def _get_nc():
    global _COMPILED_NC
    if _COMPILED_NC is None:
        _COMPILED_NC = _build_nc()
    return _COMPILED_NC


def _make_in_maps(x, W_enc, b_enc, W_dec, b_dec):
    bf16 = ml_dtypes.bfloat16
    x = np.asarray(x, dtype=np.float32)
    W_enc = np.asarray(W_enc, dtype=np.float32)
    b_enc = np.asarray(b_enc, dtype=np.float32)
    W_dec = np.asarray(W_dec, dtype=np.float32)
    b_dec = np.asarray(b_dec, dtype=np.float32)

    # x packed so each encode DMA is one contiguous [128 x 3KB] block:
    # xtp[l, h, q, p, k, b'] = x[l, h*EH + q*EH/2 + b', k*128 + p]
    xtp = np.ascontiguousarray(
        x.transpose(0, 2, 1)                      # [L, D, B]
        .reshape(L, DT, 128, B // EH, 2, EH // 2)  # [l, k, p, h, q, b']
        .transpose(0, 3, 4, 2, 1, 5)).astype(bf16)
    in_maps = []
    for i in range(NCORES):
        sl = slice(i * FL, (i + 1) * FL)
        # wencp[l, q, p, k', f] = W_enc[l, f_global, (q*3+k')*128+p]
        wencp_i = np.ascontiguousarray(
            W_enc[:, sl, :].transpose(0, 2, 1)     # [L, D, FL]
            .reshape(L, 2, DT // 2, 128, FL)       # [l, q, k', p, f]
            .transpose(0, 1, 3, 2, 4)).astype(bf16)
        benc_half = (b_enc[:, sl].reshape(L, AF, 128).transpose(2, 0, 1)
                     .reshape(128, L * AF)).astype(np.float32)
        benc_i = np.ascontiguousarray(
            np.concatenate([benc_half, benc_half * F_SCALE], axis=1))
        wd16 = W_dec[:, sl, :, :].astype(bf16).astype(np.float32)
        wdl = wd16.reshape(L, AF, 128, L, D)       # [l, a, p, j, d]
        # wdecb[c, l, j, p, a, dc] contiguous per (c, l, j)
        wdecb_i = np.ascontiguousarray(
            wdl.reshape(L, AF, 128, L, NBDC, BDC)
            .transpose(4, 0, 3, 2, 1, 5)).astype(bf16)
        # wdect[t, l, j, p, a, dc] contiguous per (t, l, j), fp8 e3m4
        wdect_i = np.ascontiguousarray(
            (wdl * W_SCALE).reshape(L, AF, 128, L, NTDC, TDC)
            .transpose(4, 0, 3, 2, 1, 5)).astype(
                ml_dtypes.float8_e3m4)
        # decoder bias, pre-RS: core i contributes b_dec[i] to its own
        # layer-i partial only
        bdec_i = np.zeros((L, 128, D), dtype=bf16)
        bdec_i[i, :, :] = b_dec[i][None, :].astype(bf16)
        in_maps.append({"xtp": xtp, "wencp": wencp_i, "benc": benc_i,
                        "wdecb": wdecb_i, "wdect": wdect_i,
                        "bdec": bdec_i})
    return in_maps


def run(x, W_enc, b_enc, W_dec, b_dec, trace=False):
    """Run the kernel; returns (output [L, B, D] fp32, BassKernelResults)."""
    from concourse import bass_utils

    nc = _get_nc()
    in_maps = _make_in_maps(x, W_enc, b_enc, W_dec, b_dec)
    res = bass_utils.run_bass_kernel_spmd(
        nc, in_maps, core_ids=list(range(NCORES)), trace=trace)
    outs = np.stack([res.results[i]["out"] for i in range(NCORES)], axis=0)
    return np.ascontiguousarray(outs.astype(np.float32)), res


def kernel(x, W_enc, b_enc, W_dec, b_dec):
    out, _ = run(x, W_enc, b_enc, W_dec, b_dec)
    return out
